# revision 13
# baseline (speedup 1.0000x reference)
"""Trainium2 Bass kernel for nn_MultiHeadAttention (B=2,S=2048,D=1024,H=16,dk=dv=64).

Sharding: 8 cores = 2 batches x 4 head-groups (4 heads/core).
Per core:
  - projections q/k in transposed [dk, S] layout (f32r matmuls, full PE rate),
    v in natural [S, dv] layout (bf16 for the context matmul).
  - scores = qT.T @ kT per head (K=64), additive mask injected via an
    identity matmul into the same PSUM accumulation group.
  - softmax: ACT exp with fused row-sum (accum_out); normalize on GPSIMD
    (attn = E * recip); attn tile DMA'd straight out.
  - context: PE-transpose normalized attn tiles (f32r, 1.5cyc/row), cast to
    bf16 on PSUM->SBUF evac, then V-stationary bf16 matmuls -> context^T.
  - out projection from context^T (f32r), + (bv@Wo + bo/4) bias, partials
    ReduceScatter'd (add) over the 4 cores of the batch, residual + LayerNorm
    on 512 rows per core.
Host only shards/gathers (transpose/slice of inputs, concat of outputs).
"""

import sys

sys.path.insert(0, "/opt/trn_rl_repo")

import numpy as np

import concourse.bass as bass
import concourse.mybir as mybir
from concourse.tile import TileContext
from concourse.bass_utils import run_bass_kernel_spmd
from concourse.masks import make_identity
from concourse.vector_clock import ScopedClock

B, S, D, H, DK, DV = 2, 2048, 1024, 16, 64, 64
NEG = -1e9
EPS = 1e-5
NCORES = 8
HPC = H // 4  # heads per core = 4
ROWS = S // 4  # output rows per core = 512

F32 = mybir.dt.float32
F32R = mybir.dt.float32r
BF16 = mybir.dt.bfloat16
ALU = mybir.AluOpType
ACTF = mybir.ActivationFunctionType


def _patch_tile_drain():
    """walrus CoreV3 codegen allows a single sync-wait command on a Drain/NOP
    (CTRL_NO) instruction; TileContext's exit drain carries one wait per
    outstanding logical processor.  Keep one wait on the drain and move the
    rest onto chained sync NOPs (same engine, program order precedes the
    all-engine barrier, so semantics are preserved)."""
    import concourse.tile as ctile

    def _drain_and_barrier(self, tick_clock, wait_clock):
        drain_inst = self.nc.sync.drain()
        wait_clock.add_sem_waits(
            drain_inst.ins, ScopedClock({None: tick_clock.global_clock})
        )
        si = drain_inst.ins.sync_info
        if si is not None and si.on_wait is not None and len(si.on_wait) > 1:
            waits = list(si.on_wait)
            si.on_wait = [waits[0]]
            for w in waits[1:]:
                nop = self.nc.sync.nop(nofuse=True, hint="drain_wait_split")
                nop.ins.sync_info = mybir.SyncInfo(on_wait=[w], on_update=[])

        self.nc.all_engine_barrier()
        assert self.sems is not None
        popped = self.nc._tile_sem_poison_stack.pop()
        assert popped is self._sem_poison
        self.nc.clear_and_free_semaphores(list(self.sems.allocated().values()))
        self.nc.all_engine_barrier()

    ctile.TileContext._drain_and_barrier = _drain_and_barrier


def _split_multi_waits(nc: bass.Bass):
    """walrus CoreV3 codegen supports only ONE sync-wait command per
    instruction (any struct).  For every instruction carrying N>1 waits,
    keep the last wait and hoist the other N-1 onto same-engine NOPs
    inserted immediately before it (in-order engine queues make this
    semantically identical)."""
    uid = [0]
    for fn in nc.m.functions:
        for blk in fn.blocks:
            insts = blk.instructions
            new_list = []
            for inst in insts:
                si = inst.sync_info
                waits = list(si.on_wait) if si is not None and si.on_wait else []
                if len(waits) > 1:
                    for w in waits[:-1]:
                        uid[0] += 1
                        nop = mybir.InstNoOp(
                            name=f"waitsplit-{uid[0]}", ins=[], outs=[]
                        )
                        nop.engine = inst.engine
                        nop.bass_nofuse = True
                        nop.sync_info = mybir.SyncInfo(on_wait=[w], on_update=[])
                        new_list.append(nop)
                        nc.register_instruction(nop, overwrite=True)
                    si.on_wait = [waits[-1]]
                new_list.append(inst)
            if len(new_list) != len(insts):
                insts[:] = new_list


def build_kernel() -> bass.Bass:
    _patch_tile_drain()
    nc = bass.Bass(num_devices=NCORES)

    # ---- per-core DRAM I/O ----
    qt_d = nc.dram_tensor("qt", [D, S], F32R, kind="ExternalInput")
    kt_d = nc.dram_tensor("kt", [D, S], F32R, kind="ExternalInput")
    vt_d = nc.dram_tensor("vt", [D, S], F32R, kind="ExternalInput")
    mask_d = nc.dram_tensor("maskf", [S, S], F32R, kind="ExternalInput")
    wq_d = nc.dram_tensor("wq", [D, 256], F32R, kind="ExternalInput")
    wk_d = nc.dram_tensor("wk", [D, 256], F32R, kind="ExternalInput")
    wv_d = nc.dram_tensor("wv", [D, 256], F32R, kind="ExternalInput")
    wo_d = nc.dram_tensor("wo", [256, D], F32R, kind="ExternalInput")
    bq_d = nc.dram_tensor("bq", [256], F32, kind="ExternalInput")
    bk_d = nc.dram_tensor("bk", [256], F32, kind="ExternalInput")
    bo_d = nc.dram_tensor("bo_rep", [128, D], F32, kind="ExternalInput")
    res_d = nc.dram_tensor("resid", [ROWS, D], F32, kind="ExternalInput")
    gam_d = nc.dram_tensor("gamma_rep", [128, D], F32, kind="ExternalInput")
    bet_d = nc.dram_tensor("beta_rep", [128, D], F32, kind="ExternalInput")

    attn_d = nc.dram_tensor("attn4", [HPC, S, S], F32R, kind="ExternalOutput")
    out_d = nc.dram_tensor("outln", [ROWS, D], F32, kind="ExternalOutput")

    import os as _os
    _dbg = _os.environ.get("KERNEL_DEBUG_TAPS") == "1"
    if _dbg:
        part_d = nc.dram_tensor("part", [S, D], F32, kind="ExternalOutput")
        rsout_d = nc.dram_tensor("rsout", [ROWS, D], F32, kind="ExternalOutput")
        ctxt_d = nc.dram_tensor("ctxt_dbg", [2, 128, S], F32R, kind="ExternalOutput")
    else:
        part_d = nc.dram_tensor("part", [S, D], F32)
        rsout_d = nc.dram_tensor("rsout", [ROWS, D], F32)

    with TileContext(nc) as tc:
        # ---------- persistent tiles ----------
        with (
            tc.tile_pool(name="const", bufs=1) as constp,
            tc.tile_pool(name="persist", bufs=1) as pers,
        ):
            ident_f32 = constp.tile([128, 128], F32)
            make_identity(nc, ident_f32[:])
            ident_r = constp.tile([128, 128], F32R)
            nc.vector.tensor_copy(ident_r[:], ident_f32[:])
            ident = ident_r[:]

            # projected activations (live across phases)
            qT = [pers.tile([128, S], F32R, tag=f"qT{p}", name=f"qT{p}") for p in range(2)]
            kT = [pers.tile([128, S], F32R, tag=f"kT{p}", name=f"kT{p}") for p in range(2)]
            v_all = pers.tile([128, 16, 256], BF16, tag="v_all")
            ctxT = [pers.tile([128, S], F32R, tag=f"ctxT{p}", name=f"ctxT{p}") for p in range(2)]

            # ---------- phase A: projections ----------
            with (
                tc.tile_pool(name="wts", bufs=1) as wpool,
                tc.tile_pool(name="qkv_in", bufs=2) as inp,
                tc.tile_pool(name="proj_ps", bufs=2, space="PSUM") as pps,
                tc.tile_pool(name="projv_ps", bufs=2, space="PSUM") as pvs,
            ):
                wq_sb = wpool.tile([128, 8, 256], F32R, tag="wq")
                wk_sb = wpool.tile([128, 8, 256], F32R, tag="wk")
                wv_sb = wpool.tile([128, 8, 256], F32R, tag="wv")
                nc.sync.dma_start(out=wq_sb[:], in_=wq_d.rearrange("(c p) j -> p c j", p=128))
                nc.sync.dma_start(out=wk_sb[:], in_=wk_d.rearrange("(c p) j -> p c j", p=128))
                nc.sync.dma_start(out=wv_sb[:], in_=wv_d.rearrange("(c p) j -> p c j", p=128))
                bq_sb = wpool.tile([128, 2], F32, tag="bq")
                bk_sb = wpool.tile([128, 2], F32, tag="bk")
                nc.sync.dma_start(out=bq_sb[:], in_=bq_d.rearrange("(pr p) -> p pr", p=128))
                nc.sync.dma_start(out=bk_sb[:], in_=bk_d.rearrange("(pr p) -> p pr", p=128))

                qt_r = qt_d.rearrange("(c p) s -> p c s", p=128)
                kt_r = kt_d.rearrange("(c p) s -> p c s", p=128)
                vt_r = vt_d.rearrange("(c p) s -> p c s", p=128)

                for s4 in range(4):
                    sl = bass.ts(s4, 512)
                    qts = inp.tile([128, 8, 512], F32R, tag="qts")
                    kts = inp.tile([128, 8, 512], F32R, tag="kts")
                    vts = inp.tile([128, 8, 512], F32R, tag="vts")
                    nc.sync.dma_start(out=qts[:], in_=qt_r[:, :, sl])
                    nc.sync.dma_start(out=kts[:], in_=kt_r[:, :, sl])
                    nc.sync.dma_start(out=vts[:], in_=vt_r[:, :, sl])

                    for pr in range(2):
                        prs = bass.ts(pr, 128)
                        psq = pps.tile([128, 512], F32, tag="psq")
                        for c in range(8):
                            nc.tensor.matmul(
                                psq[:], wq_sb[:, c, prs], qts[:, c, :],
                                start=(c == 0), stop=(c == 7),
                            )
                        nc.scalar.activation(
                            qT[pr][:, sl], psq[:], ACTF.Identity,
                            bias=bq_sb[:, pr : pr + 1],
                        )
                        psk = pps.tile([128, 512], F32, tag="psq")
                        for c in range(8):
                            nc.tensor.matmul(
                                psk[:], wk_sb[:, c, prs], kts[:, c, :],
                                start=(c == 0), stop=(c == 7),
                            )
                        nc.scalar.activation(
                            kT[pr][:, sl], psk[:], ACTF.Identity,
                            bias=bk_sb[:, pr : pr + 1],
                        )

                    for st in range(4):
                        psv = pvs.tile([128, 256], F32, tag="psv")
                        for c in range(8):
                            nc.tensor.matmul(
                                psv[:], vts[:, c, bass.ts(st, 128)], wv_sb[:, c, :],
                                start=(c == 0), stop=(c == 7),
                            )
                        nc.vector.tensor_copy(v_all[:, s4 * 4 + st, :], psv[:])

            # ---------- phase B: attention ----------
            with (
                tc.tile_pool(name="maskp", bufs=2) as maskp,
                tc.tile_pool(name="epool", bufs=2) as epool,
                tc.tile_pool(name="apool", bufs=2) as apool,
                tc.tile_pool(name="atpool", bufs=2) as atpool,
                tc.tile_pool(name="small", bufs=4) as smallp,
                tc.tile_pool(name="score_ps", bufs=1, space="PSUM") as sps,
                tc.tile_pool(name="at_ps", bufs=2, space="PSUM") as atps_pool,
                tc.tile_pool(name="ctx_ps", bufs=2, space="PSUM") as cps_pool,
            ):
                for qt in range(16):
                    qsl = bass.ts(qt, 128)
                    m_sb = maskp.tile([128, S], F32R, tag="m_sb")
                    nc.sync.dma_start(out=m_sb[:], in_=mask_d[qt * 128 : qt * 128 + 128, :])
                    for h in range(HPC):
                        pr, sub = h // 2, h % 2
                        hsl = slice(sub * 64, sub * 64 + 64)
                        ps = sps.tile([128, S], F32, tag="scores")
                        for ks in range(4):
                            ksl = bass.ts(ks, 512)
                            nc.tensor.matmul(
                                ps[:, ksl], qT[pr][hsl, qsl], kT[pr][hsl, ksl],
                                start=True, stop=False,
                            )
                            nc.tensor.matmul(
                                ps[:, ksl], ident[:], m_sb[:, ksl],
                                start=False, stop=True,
                            )
                        E = epool.tile([128, S], F32, tag="E")
                        rs = smallp.tile([128, 1], F32, tag="rs")
                        nc.scalar.activation(E[:], ps[:], ACTF.Exp, accum_out=rs[:])
                        rc = smallp.tile([128, 1], F32, tag="rc")
                        nc.vector.reciprocal(rc[:], rs[:])
                        A = apool.tile([128, S], F32R, tag="A")
                        nc.gpsimd.tensor_scalar(
                            out=A[:], in0=E[:], scalar1=rc[:], scalar2=None,
                            op0=ALU.mult,
                        )
                        nc.sync.dma_start(out=attn_d[h, qt * 128 : qt * 128 + 128, :], in_=A[:])

                        AT = atpool.tile([128, 16, 128], BF16, tag="AT")
                        for tg in range(4):
                            atp = atps_pool.tile([128, 512], F32R, tag="atps")
                            for ti in range(4):
                                tt = tg * 4 + ti
                                nc.tensor.transpose(
                                    atp[:, bass.ts(ti, 128)], A[:, bass.ts(tt, 128)], ident[:]
                                )
                            nc.any.tensor_copy(
                                AT[:, tg * 4 : tg * 4 + 4, :],
                                atp[:].rearrange("p (t f) -> p t f", t=4),
                            )
                        cps = cps_pool.tile([64, 128], F32, tag="cps")
                        vsl = slice(h * 64, h * 64 + 64)
                        for kc in range(16):
                            nc.tensor.matmul(
                                cps[:], v_all[:, kc, vsl], AT[:, kc, :],
                                start=(kc == 0), stop=(kc == 15),
                            )
                        nc.vector.tensor_copy(ctxT[pr][hsl, qsl], cps[:])

            if _dbg:
                for pr in range(2):
                    nc.sync.dma_start(out=ctxt_d[pr], in_=ctxT[pr][:])

            # ---------- phase C: output projection ----------
            with (
                tc.tile_pool(name="wo", bufs=1) as wop,
                tc.tile_pool(name="osb", bufs=3) as osb,
                tc.tile_pool(name="out_ps", bufs=2, space="PSUM") as ops_pool,
            ):
                wo_sb = wop.tile([128, 2, D], F32R, tag="wo")
                nc.sync.dma_start(out=wo_sb[:], in_=wo_d.rearrange("(pr p) m -> p pr m", p=128))
                bo_sb = wop.tile([128, D], F32, tag="bo")
                nc.sync.dma_start(out=bo_sb[:], in_=bo_d[:])

                for st in range(16):
                    ssl = bass.ts(st, 128)
                    ops = ops_pool.tile([128, D], F32, tag="ops")
                    for pr in range(2):
                        for nn2 in range(2):
                            nsl = bass.ts(nn2, 512)
                            nc.tensor.matmul(
                                ops[:, nsl], ctxT[pr][:, ssl], wo_sb[:, pr, nsl],
                                start=(pr == 0), stop=(pr == 1),
                            )
                    o_sb = osb.tile([128, D], F32, tag="o_sb")
                    nc.vector.scalar_tensor_tensor(
                        out=o_sb[:], in0=ops[:], scalar=1.0, in1=bo_sb[:],
                        op0=ALU.mult, op1=ALU.add,
                    )
                    nc.sync.dma_start(out=part_d[st * 128 : st * 128 + 128, :], in_=o_sb[:])

            # ---------- phase D: reduce-scatter + residual + layernorm ----------
            nc.gpsimd.collective_compute(
                "ReduceScatter",
                ALU.add,
                replica_groups=[[0, 1, 2, 3], [4, 5, 6, 7]],
                ins=[part_d[:]],
                outs=[rsout_d[:]],
            )
            with (
                tc.tile_pool(name="ln", bufs=2) as lnp,
                tc.tile_pool(name="lnc", bufs=1) as lnc,
            ):
                gam_sb = lnc.tile([128, D], F32, tag="gam")
                bet_sb = lnc.tile([128, D], F32, tag="bet")
                nc.sync.dma_start(out=gam_sb[:], in_=gam_d[:])
                nc.sync.dma_start(out=bet_sb[:], in_=bet_d[:])
                for st in range(4):
                    rsl = slice(st * 128, st * 128 + 128)
                    x = lnp.tile([128, D], F32, tag="x")
                    r = lnp.tile([128, D], F32, tag="r")
                    nc.sync.dma_start(out=x[:], in_=rsout_d[rsl, :])
                    nc.sync.dma_start(out=r[:], in_=res_d[rsl, :])
                    xr = lnp.tile([128, D], F32, tag="xr")
                    nc.vector.tensor_add(xr[:], x[:], r[:])
                    s1 = lnp.tile([128, 1], F32, tag="s1")
                    nc.vector.tensor_reduce(s1[:], xr[:], mybir.AxisListType.X, ALU.add)
                    xsq = lnp.tile([128, D], F32, tag="xsq")
                    s2 = lnp.tile([128, 1], F32, tag="s2")
                    nc.scalar.activation(xsq[:], xr[:], ACTF.Square, accum_out=s2[:])
                    mu = lnp.tile([128, 1], F32, tag="mu")
                    nc.vector.tensor_scalar_mul(mu[:], s1[:], 1.0 / D)
                    ex2 = lnp.tile([128, 1], F32, tag="ex2")
                    nc.vector.tensor_scalar_mul(ex2[:], s2[:], 1.0 / D)
                    mu2 = lnp.tile([128, 1], F32, tag="mu2")
                    nc.vector.tensor_mul(mu2[:], mu[:], mu[:])
                    var = lnp.tile([128, 1], F32, tag="var")
                    nc.vector.tensor_sub(var[:], ex2[:], mu2[:])
                    vareps = lnp.tile([128, 1], F32, tag="vareps")
                    nc.vector.tensor_scalar_add(vareps[:], var[:], EPS)
                    std = lnp.tile([128, 1], F32, tag="std")
                    nc.scalar.activation(std[:], vareps[:], ACTF.Sqrt)
                    rstd = lnp.tile([128, 1], F32, tag="rstd")
                    nc.vector.reciprocal(rstd[:], std[:])
                    xh = lnp.tile([128, D], F32, tag="xh")
                    nc.vector.tensor_scalar(
                        out=xh[:], in0=xr[:], scalar1=mu[:], scalar2=rstd[:],
                        op0=ALU.subtract, op1=ALU.mult,
                    )
                    y1 = lnp.tile([128, D], F32, tag="y1")
                    nc.vector.tensor_mul(y1[:], xh[:], gam_sb[:])
                    y = lnp.tile([128, D], F32, tag="y")
                    nc.vector.tensor_add(y[:], y1[:], bet_sb[:])
                    nc.sync.dma_start(out=out_d[rsl, :], in_=y[:])

    _split_multi_waits(nc)
    return nc


_NC_CACHE = {}


def _get_nc() -> bass.Bass:
    if "nc" not in _NC_CACHE:
        _NC_CACHE["nc"] = build_kernel()
    return _NC_CACHE["nc"]


def _shard_inputs(Q, K, V, attn_mask, Wq, bq, Wk, bk, Wv, bv, Wo, bo, gamma, beta):
    in_maps = []
    f32 = np.float32
    for c in range(NCORES):
        b, g = c // 4, c % 4
        cols = slice(g * 256, (g + 1) * 256)
        wq_c = np.ascontiguousarray(Wq[:, cols] / 8.0, dtype=f32)
        bq_c = np.ascontiguousarray(bq[cols] / 8.0, dtype=f32)
        wk_c = np.ascontiguousarray(Wk[:, cols], dtype=f32)
        bk_c = np.ascontiguousarray(bk[cols], dtype=f32)
        wv_c = np.ascontiguousarray(Wv[:, cols], dtype=f32)
        bv_c = np.ascontiguousarray(bv[cols], dtype=f32)
        wo_c = np.ascontiguousarray(Wo[g * 256 : (g + 1) * 256, :], dtype=f32)
        bo_eff = (bv_c @ wo_c + bo.astype(f32) / 4.0).astype(f32)
        in_maps.append(
            {
                "qt": np.ascontiguousarray(Q[b].T, dtype=f32),
                "kt": np.ascontiguousarray(K[b].T, dtype=f32),
                "vt": np.ascontiguousarray(V[b].T, dtype=f32),
                "maskf": np.where(attn_mask[b], f32(NEG), f32(0.0)).astype(f32),
                "wq": wq_c,
                "bq": bq_c,
                "wk": wk_c,
                "bk": bk_c,
                "wv": wv_c,
                "wo": wo_c,
                "bo_rep": np.broadcast_to(bo_eff, (128, D)).copy(),
                "resid": np.ascontiguousarray(Q[b, g * ROWS : (g + 1) * ROWS, :], dtype=f32),
                "gamma_rep": np.broadcast_to(gamma.astype(f32), (128, D)).copy(),
                "beta_rep": np.broadcast_to(beta.astype(f32), (128, D)).copy(),
            }
        )
    return in_maps


def _run(in_maps, **kwargs):
    nc = _get_nc()
    return run_bass_kernel_spmd(nc, in_maps, core_ids=list(range(NCORES)), **kwargs)


def kernel(Q, K, V, attn_mask, Wq, bq, Wk, bk, Wv, bv, Wo, bo, gamma, beta, _res_out=None, _run_kwargs=None):
    args = (Q, K, V, attn_mask, Wq, bq, Wk, bk, Wv, bv, Wo, bo, gamma, beta)
    args = tuple(np.asarray(a) for a in args)
    in_maps = _shard_inputs(*args)
    res = _run(in_maps, **(_run_kwargs or {}))
    if _res_out is not None:
        _res_out.append(res)

    out = np.empty((B, S, D), np.float32)
    attn = np.empty((B, H, S, S), np.float32)
    for c in range(NCORES):
        b, g = c // 4, c % 4
        out[b, g * ROWS : (g + 1) * ROWS, :] = res.results[c]["outln"]
        attn[b, 4 * g : 4 * g + 4] = res.results[c]["attn4"]
    return out, attn


# revision 14
# speedup vs baseline: 3.0195x; 3.0195x over previous
"""Trainium2 Bass kernel for nn_MultiHeadAttention (B=2,S=2048,D=1024,H=16,dk=dv=64).

Sharding: 8 cores = 2 batches x 4 head-groups (4 heads/core).
Per core:
  - projections q/k in transposed [dk, S] layout (f32r matmuls, full PE rate),
    v in natural [S, dv] layout (bf16 for the context matmul).
  - scores = qT.T @ kT per head (K=64), additive mask injected via an
    identity matmul into the same PSUM accumulation group.
  - softmax: ACT exp with fused row-sum (accum_out); normalize on GPSIMD
    (attn = E * recip); attn tile DMA'd straight out.
  - context: PE-transpose normalized attn tiles (f32r, 1.5cyc/row), cast to
    bf16 on PSUM->SBUF evac, then V-stationary bf16 matmuls -> context^T.
  - out projection from context^T (f32r), + (bv@Wo + bo/4) bias, partials
    ReduceScatter'd (add) over the 4 cores of the batch, residual + LayerNorm
    on 512 rows per core.
Host only shards/gathers (transpose/slice of inputs, concat of outputs).
"""

import sys

sys.path.insert(0, "/opt/trn_rl_repo")

import numpy as np

import concourse.bass as bass
import concourse.mybir as mybir
from concourse.tile import TileContext
from concourse.bass_utils import run_bass_kernel_spmd
from concourse.masks import make_identity
from concourse.vector_clock import ScopedClock

B, S, D, H, DK, DV = 2, 2048, 1024, 16, 64, 64
NEG = -1e9
EPS = 1e-5
NCORES = 8
HPC = H // 4  # heads per core = 4
ROWS = S // 4  # output rows per core = 512

F32 = mybir.dt.float32
F32R = mybir.dt.float32r
BF16 = mybir.dt.bfloat16
ALU = mybir.AluOpType
ACTF = mybir.ActivationFunctionType


def _patch_tile_drain():
    """walrus CoreV3 codegen allows a single sync-wait command on a Drain/NOP
    (CTRL_NO) instruction; TileContext's exit drain carries one wait per
    outstanding logical processor.  Keep one wait on the drain and move the
    rest onto chained sync NOPs (same engine, program order precedes the
    all-engine barrier, so semantics are preserved)."""
    import concourse.tile as ctile

    def _drain_and_barrier(self, tick_clock, wait_clock):
        drain_inst = self.nc.sync.drain()
        wait_clock.add_sem_waits(
            drain_inst.ins, ScopedClock({None: tick_clock.global_clock})
        )
        si = drain_inst.ins.sync_info
        if si is not None and si.on_wait is not None and len(si.on_wait) > 1:
            waits = list(si.on_wait)
            si.on_wait = [waits[0]]
            for w in waits[1:]:
                nop = self.nc.sync.nop(nofuse=True, hint="drain_wait_split")
                nop.ins.sync_info = mybir.SyncInfo(on_wait=[w], on_update=[])

        self.nc.all_engine_barrier()
        assert self.sems is not None
        popped = self.nc._tile_sem_poison_stack.pop()
        assert popped is self._sem_poison
        self.nc.clear_and_free_semaphores(list(self.sems.allocated().values()))
        self.nc.all_engine_barrier()

    ctile.TileContext._drain_and_barrier = _drain_and_barrier


def _split_multi_waits(nc: bass.Bass):
    """walrus CoreV3 codegen supports only ONE sync-wait command per
    instruction (any struct).  For every instruction carrying N>1 waits,
    keep the last wait and hoist the other N-1 onto same-engine NOPs
    inserted immediately before it (in-order engine queues make this
    semantically identical)."""
    uid = [0]
    for fn in nc.m.functions:
        for blk in fn.blocks:
            insts = blk.instructions
            new_list = []
            for inst in insts:
                si = inst.sync_info
                waits = list(si.on_wait) if si is not None and si.on_wait else []
                if len(waits) > 1:
                    for w in waits[:-1]:
                        uid[0] += 1
                        nop = mybir.InstNoOp(
                            name=f"waitsplit-{uid[0]}", ins=[], outs=[]
                        )
                        nop.engine = inst.engine
                        nop.bass_nofuse = True
                        nop.sync_info = mybir.SyncInfo(on_wait=[w], on_update=[])
                        new_list.append(nop)
                        nc.register_instruction(nop, overwrite=True)
                    si.on_wait = [waits[-1]]
                new_list.append(inst)
            if len(new_list) != len(insts):
                insts[:] = new_list


def build_kernel() -> bass.Bass:
    _patch_tile_drain()
    nc = bass.Bass(num_devices=NCORES)

    # ---- per-core DRAM I/O ----
    qt_d = nc.dram_tensor("qt", [D, S], F32R, kind="ExternalInput")
    kt_d = nc.dram_tensor("kt", [D, S], F32R, kind="ExternalInput")
    vt_d = nc.dram_tensor("vt", [D, S], F32R, kind="ExternalInput")
    mask_d = nc.dram_tensor("maskf", [S, S], F32R, kind="ExternalInput")
    wq_d = nc.dram_tensor("wq", [D, 256], F32R, kind="ExternalInput")
    wk_d = nc.dram_tensor("wk", [D, 256], F32R, kind="ExternalInput")
    wv_d = nc.dram_tensor("wv", [D, 256], F32R, kind="ExternalInput")
    wo_d = nc.dram_tensor("wo", [256, D], F32R, kind="ExternalInput")
    bq_d = nc.dram_tensor("bq", [256], F32, kind="ExternalInput")
    bk_d = nc.dram_tensor("bk", [256], F32, kind="ExternalInput")
    bo_d = nc.dram_tensor("bo_rep", [128, D], F32, kind="ExternalInput")
    res_d = nc.dram_tensor("resid", [ROWS, D], F32, kind="ExternalInput")
    gam_d = nc.dram_tensor("gamma_rep", [128, D], F32, kind="ExternalInput")
    bet_d = nc.dram_tensor("beta_rep", [128, D], F32, kind="ExternalInput")

    attn_d = nc.dram_tensor("attn4", [HPC, S, S], F32R, kind="ExternalOutput")
    out_d = nc.dram_tensor("outln", [ROWS, D], F32, kind="ExternalOutput")

    import os as _os
    _dbg = _os.environ.get("KERNEL_DEBUG_TAPS") == "1"
    if _dbg:
        part_d = nc.dram_tensor("part", [S, D], F32, kind="ExternalOutput")
        rsout_d = nc.dram_tensor("rsout", [ROWS, D], F32, kind="ExternalOutput")
        ctxt_d = nc.dram_tensor("ctxt_dbg", [2, 128, S], F32R, kind="ExternalOutput")
    else:
        part_d = nc.dram_tensor("part", [S, D], F32)
        rsout_d = nc.dram_tensor("rsout", [ROWS, D], F32)

    with TileContext(nc) as tc:
        # ---------- persistent tiles ----------
        with (
            tc.tile_pool(name="const", bufs=1) as constp,
            tc.tile_pool(name="persist", bufs=1) as pers,
        ):
            ident_f32 = constp.tile([128, 128], F32)
            make_identity(nc, ident_f32[:])
            ident_r = constp.tile([128, 128], F32R)
            nc.vector.tensor_copy(ident_r[:], ident_f32[:])
            ident = ident_r[:]

            # projected activations (live across phases)
            qT = [pers.tile([128, S], F32R, tag=f"qT{p}", name=f"qT{p}") for p in range(2)]
            kT = [pers.tile([128, S], F32R, tag=f"kT{p}", name=f"kT{p}") for p in range(2)]
            v_all = pers.tile([128, 16, 256], BF16, tag="v_all")
            ctxT = [pers.tile([128, S], F32R, tag=f"ctxT{p}", name=f"ctxT{p}") for p in range(2)]

            # ---------- phase A: projections ----------
            with (
                tc.tile_pool(name="wts", bufs=1) as wpool,
                tc.tile_pool(name="qkv_in", bufs=2) as inp,
                tc.tile_pool(name="proj_ps", bufs=2, space="PSUM") as pps,
                tc.tile_pool(name="projv_ps", bufs=2, space="PSUM") as pvs,
            ):
                wq_sb = wpool.tile([128, 8, 256], F32R, tag="wq")
                wk_sb = wpool.tile([128, 8, 256], F32R, tag="wk")
                wv_sb = wpool.tile([128, 8, 256], F32R, tag="wv")
                nc.sync.dma_start(out=wq_sb[:], in_=wq_d.rearrange("(c p) j -> p c j", p=128))
                nc.sync.dma_start(out=wk_sb[:], in_=wk_d.rearrange("(c p) j -> p c j", p=128))
                nc.sync.dma_start(out=wv_sb[:], in_=wv_d.rearrange("(c p) j -> p c j", p=128))
                bq_sb = wpool.tile([128, 2], F32, tag="bq")
                bk_sb = wpool.tile([128, 2], F32, tag="bk")
                nc.sync.dma_start(out=bq_sb[:], in_=bq_d.rearrange("(pr p) -> p pr", p=128))
                nc.sync.dma_start(out=bk_sb[:], in_=bk_d.rearrange("(pr p) -> p pr", p=128))

                qt_r = qt_d.rearrange("(c p) s -> p c s", p=128)
                kt_r = kt_d.rearrange("(c p) s -> p c s", p=128)
                vt_r = vt_d.rearrange("(c p) s -> p c s", p=128)

                for s4 in range(4):
                    sl = bass.ts(s4, 512)
                    qts = inp.tile([128, 8, 512], F32R, tag="qts")
                    kts = inp.tile([128, 8, 512], F32R, tag="kts")
                    vts = inp.tile([128, 8, 512], F32R, tag="vts")
                    nc.sync.dma_start(out=qts[:], in_=qt_r[:, :, sl])
                    nc.sync.dma_start(out=kts[:], in_=kt_r[:, :, sl])
                    nc.sync.dma_start(out=vts[:], in_=vt_r[:, :, sl])

                    for pr in range(2):
                        prs = bass.ts(pr, 128)
                        psq = pps.tile([128, 512], F32, tag="psq")
                        for c in range(8):
                            nc.tensor.matmul(
                                psq[:], wq_sb[:, c, prs], qts[:, c, :],
                                start=(c == 0), stop=(c == 7),
                            )
                        nc.scalar.activation(
                            qT[pr][:, sl], psq[:], ACTF.Identity,
                            bias=bq_sb[:, pr : pr + 1],
                        )
                        psk = pps.tile([128, 512], F32, tag="psq")
                        for c in range(8):
                            nc.tensor.matmul(
                                psk[:], wk_sb[:, c, prs], kts[:, c, :],
                                start=(c == 0), stop=(c == 7),
                            )
                        nc.scalar.activation(
                            kT[pr][:, sl], psk[:], ACTF.Identity,
                            bias=bk_sb[:, pr : pr + 1],
                        )

                    for st in range(4):
                        psv = pvs.tile([128, 256], F32, tag="psv")
                        for c in range(8):
                            nc.tensor.matmul(
                                psv[:], vts[:, c, bass.ts(st, 128)], wv_sb[:, c, :],
                                start=(c == 0), stop=(c == 7),
                            )
                        nc.vector.tensor_copy(v_all[:, s4 * 4 + st, :], psv[:])

            # ---------- phase B: attention ----------
            with (
                tc.tile_pool(name="maskp", bufs=2) as maskp,
                tc.tile_pool(name="epool", bufs=2) as epool,
                tc.tile_pool(name="apool", bufs=2) as apool,
                tc.tile_pool(name="atpool", bufs=2) as atpool,
                tc.tile_pool(name="small", bufs=4) as smallp,
                tc.tile_pool(name="score_ps", bufs=1, space="PSUM") as sps,
                tc.tile_pool(name="at_ps", bufs=2, space="PSUM") as atps_pool,
                tc.tile_pool(name="ctx_ps", bufs=2, space="PSUM") as cps_pool,
            ):
                for qt in range(16):
                    qsl = bass.ts(qt, 128)
                    m_sb = maskp.tile([128, S], F32R, tag="m_sb")
                    nc.sync.dma_start(out=m_sb[:], in_=mask_d[qt * 128 : qt * 128 + 128, :])
                    for h in range(HPC):
                        pr, sub = h // 2, h % 2
                        hsl = slice(sub * 64, sub * 64 + 64)
                        ps = sps.tile([128, S], F32, tag="scores")
                        for ks in range(4):
                            ksl = bass.ts(ks, 512)
                            nc.tensor.matmul(
                                ps[:, ksl], qT[pr][hsl, qsl], kT[pr][hsl, ksl],
                                start=True, stop=False,
                            )
                            nc.tensor.matmul(
                                ps[:, ksl], ident[:], m_sb[:, ksl],
                                start=False, stop=True,
                            )
                        E = epool.tile([128, S], F32, tag="E")
                        rs = smallp.tile([128, 1], F32, tag="rs")
                        nc.scalar.activation(E[:], ps[:], ACTF.Exp, accum_out=rs[:])
                        rc = smallp.tile([128, 1], F32, tag="rc")
                        nc.vector.reciprocal(rc[:], rs[:])
                        A = apool.tile([128, S], F32R, tag="A")
                        nc.vector.tensor_scalar(
                            out=A[:], in0=E[:], scalar1=rc[:], scalar2=None,
                            op0=ALU.mult,
                        )
                        nc.sync.dma_start(out=attn_d[h, qt * 128 : qt * 128 + 128, :], in_=A[:])

                        AT = atpool.tile([128, 16, 128], BF16, tag="AT")
                        for tg in range(4):
                            atp = atps_pool.tile([128, 512], F32R, tag="atps")
                            for ti in range(4):
                                tt = tg * 4 + ti
                                nc.tensor.transpose(
                                    atp[:, bass.ts(ti, 128)], A[:, bass.ts(tt, 128)], ident[:]
                                )
                            if tg % 2 == 0:
                                nc.vector.tensor_copy(
                                    AT[:, tg * 4 : tg * 4 + 4, :],
                                    atp[:].rearrange("p (t f) -> p t f", t=4),
                                )
                            else:
                                nc.scalar.activation(
                                    AT[:, tg * 4 : tg * 4 + 4, :],
                                    atp[:].rearrange("p (t f) -> p t f", t=4),
                                    ACTF.Copy,
                                )
                        cps = cps_pool.tile([64, 128], F32, tag="cps")
                        vsl = slice(h * 64, h * 64 + 64)
                        for kc in range(16):
                            nc.tensor.matmul(
                                cps[:], v_all[:, kc, vsl], AT[:, kc, :],
                                start=(kc == 0), stop=(kc == 15),
                            )
                        nc.vector.tensor_copy(ctxT[pr][hsl, qsl], cps[:])

            if _dbg:
                for pr in range(2):
                    nc.sync.dma_start(out=ctxt_d[pr], in_=ctxT[pr][:])

            # ---------- phase C: output projection ----------
            with (
                tc.tile_pool(name="wo", bufs=1) as wop,
                tc.tile_pool(name="osb", bufs=3) as osb,
                tc.tile_pool(name="out_ps", bufs=2, space="PSUM") as ops_pool,
            ):
                wo_sb = wop.tile([128, 2, D], F32R, tag="wo")
                nc.sync.dma_start(out=wo_sb[:], in_=wo_d.rearrange("(pr p) m -> p pr m", p=128))
                bo_sb = wop.tile([128, D], F32, tag="bo")
                nc.sync.dma_start(out=bo_sb[:], in_=bo_d[:])

                for st in range(16):
                    ssl = bass.ts(st, 128)
                    ops = ops_pool.tile([128, D], F32, tag="ops")
                    for pr in range(2):
                        for nn2 in range(2):
                            nsl = bass.ts(nn2, 512)
                            nc.tensor.matmul(
                                ops[:, nsl], ctxT[pr][:, ssl], wo_sb[:, pr, nsl],
                                start=(pr == 0), stop=(pr == 1),
                            )
                    o_sb = osb.tile([128, D], F32, tag="o_sb")
                    nc.vector.scalar_tensor_tensor(
                        out=o_sb[:], in0=ops[:], scalar=1.0, in1=bo_sb[:],
                        op0=ALU.mult, op1=ALU.add,
                    )
                    nc.sync.dma_start(out=part_d[st * 128 : st * 128 + 128, :], in_=o_sb[:])

            # ---------- phase D: reduce-scatter + residual + layernorm ----------
            nc.gpsimd.collective_compute(
                "ReduceScatter",
                ALU.add,
                replica_groups=[[0, 1, 2, 3], [4, 5, 6, 7]],
                ins=[part_d[:]],
                outs=[rsout_d[:]],
            )
            with (
                tc.tile_pool(name="ln", bufs=2) as lnp,
                tc.tile_pool(name="lnc", bufs=1) as lnc,
            ):
                gam_sb = lnc.tile([128, D], F32, tag="gam")
                bet_sb = lnc.tile([128, D], F32, tag="bet")
                nc.sync.dma_start(out=gam_sb[:], in_=gam_d[:])
                nc.sync.dma_start(out=bet_sb[:], in_=bet_d[:])
                for st in range(4):
                    rsl = slice(st * 128, st * 128 + 128)
                    x = lnp.tile([128, D], F32, tag="x")
                    r = lnp.tile([128, D], F32, tag="r")
                    nc.sync.dma_start(out=x[:], in_=rsout_d[rsl, :])
                    nc.sync.dma_start(out=r[:], in_=res_d[rsl, :])
                    xr = lnp.tile([128, D], F32, tag="xr")
                    nc.vector.tensor_add(xr[:], x[:], r[:])
                    s1 = lnp.tile([128, 1], F32, tag="s1")
                    nc.vector.tensor_reduce(s1[:], xr[:], mybir.AxisListType.X, ALU.add)
                    xsq = lnp.tile([128, D], F32, tag="xsq")
                    s2 = lnp.tile([128, 1], F32, tag="s2")
                    nc.scalar.activation(xsq[:], xr[:], ACTF.Square, accum_out=s2[:])
                    mu = lnp.tile([128, 1], F32, tag="mu")
                    nc.vector.tensor_scalar_mul(mu[:], s1[:], 1.0 / D)
                    ex2 = lnp.tile([128, 1], F32, tag="ex2")
                    nc.vector.tensor_scalar_mul(ex2[:], s2[:], 1.0 / D)
                    mu2 = lnp.tile([128, 1], F32, tag="mu2")
                    nc.vector.tensor_mul(mu2[:], mu[:], mu[:])
                    var = lnp.tile([128, 1], F32, tag="var")
                    nc.vector.tensor_sub(var[:], ex2[:], mu2[:])
                    vareps = lnp.tile([128, 1], F32, tag="vareps")
                    nc.vector.tensor_scalar_add(vareps[:], var[:], EPS)
                    std = lnp.tile([128, 1], F32, tag="std")
                    nc.scalar.activation(std[:], vareps[:], ACTF.Sqrt)
                    rstd = lnp.tile([128, 1], F32, tag="rstd")
                    nc.vector.reciprocal(rstd[:], std[:])
                    xh = lnp.tile([128, D], F32, tag="xh")
                    nc.vector.tensor_scalar(
                        out=xh[:], in0=xr[:], scalar1=mu[:], scalar2=rstd[:],
                        op0=ALU.subtract, op1=ALU.mult,
                    )
                    y1 = lnp.tile([128, D], F32, tag="y1")
                    nc.vector.tensor_mul(y1[:], xh[:], gam_sb[:])
                    y = lnp.tile([128, D], F32, tag="y")
                    nc.vector.tensor_add(y[:], y1[:], bet_sb[:])
                    nc.sync.dma_start(out=out_d[rsl, :], in_=y[:])

    _split_multi_waits(nc)
    return nc


_NC_CACHE = {}


def _get_nc() -> bass.Bass:
    if "nc" not in _NC_CACHE:
        _NC_CACHE["nc"] = build_kernel()
    return _NC_CACHE["nc"]


def _shard_inputs(Q, K, V, attn_mask, Wq, bq, Wk, bk, Wv, bv, Wo, bo, gamma, beta):
    in_maps = []
    f32 = np.float32
    for c in range(NCORES):
        b, g = c // 4, c % 4
        cols = slice(g * 256, (g + 1) * 256)
        wq_c = np.ascontiguousarray(Wq[:, cols] / 8.0, dtype=f32)
        bq_c = np.ascontiguousarray(bq[cols] / 8.0, dtype=f32)
        wk_c = np.ascontiguousarray(Wk[:, cols], dtype=f32)
        bk_c = np.ascontiguousarray(bk[cols], dtype=f32)
        wv_c = np.ascontiguousarray(Wv[:, cols], dtype=f32)
        bv_c = np.ascontiguousarray(bv[cols], dtype=f32)
        wo_c = np.ascontiguousarray(Wo[g * 256 : (g + 1) * 256, :], dtype=f32)
        bo_eff = (bv_c @ wo_c + bo.astype(f32) / 4.0).astype(f32)
        in_maps.append(
            {
                "qt": np.ascontiguousarray(Q[b].T, dtype=f32),
                "kt": np.ascontiguousarray(K[b].T, dtype=f32),
                "vt": np.ascontiguousarray(V[b].T, dtype=f32),
                "maskf": np.where(attn_mask[b], f32(NEG), f32(0.0)).astype(f32),
                "wq": wq_c,
                "bq": bq_c,
                "wk": wk_c,
                "bk": bk_c,
                "wv": wv_c,
                "wo": wo_c,
                "bo_rep": np.broadcast_to(bo_eff, (128, D)).copy(),
                "resid": np.ascontiguousarray(Q[b, g * ROWS : (g + 1) * ROWS, :], dtype=f32),
                "gamma_rep": np.broadcast_to(gamma.astype(f32), (128, D)).copy(),
                "beta_rep": np.broadcast_to(beta.astype(f32), (128, D)).copy(),
            }
        )
    return in_maps


def _run(in_maps, **kwargs):
    nc = _get_nc()
    return run_bass_kernel_spmd(nc, in_maps, core_ids=list(range(NCORES)), **kwargs)


def kernel(Q, K, V, attn_mask, Wq, bq, Wk, bk, Wv, bv, Wo, bo, gamma, beta, _res_out=None, _run_kwargs=None):
    args = (Q, K, V, attn_mask, Wq, bq, Wk, bk, Wv, bv, Wo, bo, gamma, beta)
    args = tuple(np.asarray(a) for a in args)
    in_maps = _shard_inputs(*args)
    res = _run(in_maps, **(_run_kwargs or {}))
    if _res_out is not None:
        _res_out.append(res)

    out = np.empty((B, S, D), np.float32)
    attn = np.empty((B, H, S, S), np.float32)
    for c in range(NCORES):
        b, g = c // 4, c % 4
        out[b, g * ROWS : (g + 1) * ROWS, :] = res.results[c]["outln"]
        attn[b, 4 * g : 4 * g + 4] = res.results[c]["attn4"]
    return out, attn
